# revision 4
# baseline (speedup 1.0000x reference)
# Trainium2 Bass kernel for nn_Attention3 (unnormalized linear attention).
#
# Math: e_i = x @ W_i.T + b_i (i=1,2,3);  out = sigmoid((e1 @ e2.T @ e3) @ WO.T + bO)
# Since there is no softmax, (e1 @ e2.T) @ e3 == e1 @ (e2.T @ e3) where
# KV = e2.T @ e3 is only [64, 64].  The kernel is therefore memory-bound:
# read x once, write out once.
#
# Sharding: the flattened [B*S, 512] = [16384, 512] rows are split into 8
# contiguous chunks of 2048 rows (cores 0-3 <- batch 0, cores 4-7 <- batch 1).
# Each core computes its partial KV^T = e3_c.T @ e2_c over its rows, the four
# cores of a batch AllGather+sum their partials, then each core finishes
# out = sigmoid(e1 @ (KV @ WO.T) + bO) for its rows.
#
# Host-side prep (cheap, numpy): x is passed transposed ([512, rows] f32 per
# core) so that the contraction dim (512) lands on SBUF partitions; weights
# are passed pre-transposed in bf16.  All matmuls run in bf16 with f32 PSUM
# accumulation.

import numpy as np
import ml_dtypes

import concourse.bass as bass
import concourse.mybir as mybir
import concourse.tile as tile
from concourse import bacc
from concourse.bass_utils import run_bass_kernel_spmd

BATCH = 2
SEQ = 8192
DIN = 512
DE = 64
N_CORES = 8
ROWS = (BATCH * SEQ) // N_CORES  # 2048 rows per core
BF16 = ml_dtypes.bfloat16

TRACE = False
TRACE_KWARGS = {}
LAST_RESULT = None

_NC_CACHE = {}


def build_nc(rows=ROWS, n_cores=N_CORES):
    f32 = mybir.dt.float32
    bf16 = mybir.dt.bfloat16

    half = n_cores // 2
    groups = [list(range(half)), list(range(half, n_cores))]
    ngrp = half

    assert rows % 512 == 0
    n_chunks = rows // 512
    n_tiles = rows // 128

    nc = bacc.Bacc(None, target_bir_lowering=False, debug=False, num_devices=n_cores)

    xt = nc.dram_tensor("xt", [DIN, rows], f32, kind="ExternalInput")
    w1t = nc.dram_tensor("w1t", [DIN, DE], bf16, kind="ExternalInput")
    w23t = nc.dram_tensor("w23t", [DIN, 2 * DE], bf16, kind="ExternalInput")
    wot = nc.dram_tensor("wot", [DE, DIN], bf16, kind="ExternalInput")
    b1 = nc.dram_tensor("b1", [DE, 1], f32, kind="ExternalInput")
    b23 = nc.dram_tensor("b23", [1, 2 * DE], f32, kind="ExternalInput")
    bo = nc.dram_tensor("bo", [1, DIN], bf16, kind="ExternalInput")
    out = nc.dram_tensor("out", [rows, DIN], f32, kind="ExternalOutput")

    xt_t = xt.ap().rearrange("(kt p) s -> p kt s", p=128)  # [128, 4, rows]

    with tile.TileContext(nc) as tc:
        with (
            tc.tile_pool(name="consts", bufs=1) as consts,
            tc.tile_pool(name="persist", bufs=1) as persist,
            tc.tile_pool(name="kvps", bufs=1, space="PSUM") as kvps,
            tc.tile_pool(name="dram", bufs=1, space="DRAM") as dram,
        ):
            sb_w1t = consts.tile([128, 4, DE], bf16)
            nc.sync.dma_start(out=sb_w1t, in_=w1t.ap().rearrange("(kt p) d -> p kt d", p=128))
            sb_w23t = consts.tile([128, 4, 2 * DE], bf16)
            nc.sync.dma_start(out=sb_w23t, in_=w23t.ap().rearrange("(kt p) d -> p kt d", p=128))
            sb_wot = consts.tile([DE, DIN], bf16)
            nc.sync.dma_start(out=sb_wot, in_=wot.ap())
            sb_b1 = consts.tile([DE, 1], f32)
            nc.sync.dma_start(out=sb_b1, in_=b1.ap())
            sb_b23 = consts.tile([128, 2 * DE], f32)
            nc.gpsimd.dma_start(out=sb_b23, in_=b23.ap().to_broadcast((128, 2 * DE)))

            # e1^T for all local rows, with a row of ones at partition DE so the
            # final matmul folds in the output bias (lhsT K = DE+1).
            e1t = persist.tile([128, rows], bf16)
            nc.vector.memset(e1t[DE : DE + 1, :], 1.0)
            # M = KV @ WO.T in rows 0..63, bO in row DE.
            mmat = persist.tile([128, DIN], bf16)
            nc.sync.dma_start(out=mmat[DE : DE + 1, :], in_=bo.ap())

            kvt_ps = kvps.tile([DE, DE], f32)  # accumulates e3^T @ e2 over all tiles

            # ---- Phase A: load x^T, project, accumulate partial KV^T ----
            with (
                tc.tile_pool(name="xf", bufs=2) as xfp,
                tc.tile_pool(name="xb", bufs=2) as xbp,
                tc.tile_pool(name="e23ps", bufs=2, space="PSUM") as e23psp,
                tc.tile_pool(name="e1ps", bufs=2, space="PSUM") as e1psp,
                tc.tile_pool(name="e23sb", bufs=3) as e23sbp,
            ):
                for j in range(n_chunks):
                    xf = xfp.tile([128, 4, 512], f32)
                    nc.sync.dma_start(out=xf, in_=xt_t[:, :, j * 512 : (j + 1) * 512])
                    xb = xbp.tile([128, 4, 512], bf16)
                    nc.vector.tensor_copy(xb, xf)
                    for t in range(4):
                        tt = j * 4 + t
                        sl = slice(t * 128, (t + 1) * 128)
                        e23_ps = e23psp.tile([128, 2 * DE], f32)
                        for kt in range(4):
                            nc.tensor.matmul(
                                e23_ps,
                                lhsT=xb[:, kt, sl],
                                rhs=sb_w23t[:, kt, :],
                                start=(kt == 0),
                                stop=(kt == 3),
                            )
                        e23_sb = e23sbp.tile([128, 2 * DE], bf16)
                        nc.vector.tensor_add(e23_sb, e23_ps, sb_b23)
                        nc.tensor.matmul(
                            kvt_ps,
                            lhsT=e23_sb[:, DE:],
                            rhs=e23_sb[:, :DE],
                            start=(tt == 0),
                            stop=(tt == n_tiles - 1),
                        )
                        e1_ps = e1psp.tile([DE, 128], f32)
                        for kt in range(4):
                            nc.tensor.matmul(
                                e1_ps,
                                lhsT=sb_w1t[:, kt, :],
                                rhs=xb[:, kt, sl],
                                start=(kt == 0),
                                stop=(kt == 3),
                            )
                        nc.scalar.activation(
                            e1t[:DE, tt * 128 : (tt + 1) * 128],
                            e1_ps,
                            mybir.ActivationFunctionType.Identity,
                            bias=sb_b1,
                            scale=1.0,
                        )

            # ---- Collective: AllGather partial KV^T within the batch group ----
            with (
                tc.tile_pool(name="small", bufs=1) as small,
                tc.tile_pool(name="mmps", bufs=1, space="PSUM") as mmpsp,
            ):
                kvt_sb = small.tile([DE, DE], f32)
                nc.vector.tensor_copy(kvt_sb, kvt_ps)
                cc_in = dram.tile([DE, DE], f32)
                cc_out = dram.tile([ngrp, DE, DE], f32)
                nc.gpsimd.dma_start(out=cc_in[:, :], in_=kvt_sb)
                nc.gpsimd.collective_compute(
                    "AllGather",
                    mybir.AluOpType.bypass,
                    replica_groups=groups,
                    ins=[cc_in[:, :]],
                    outs=[cc_out[:, :, :]],
                )
                kvt_all = small.tile([DE, ngrp, DE], f32)
                nc.sync.dma_start(
                    out=kvt_all, in_=cc_out[:, :, :].rearrange("r p d -> p r d")
                )
                # tree-sum the ngrp partials, final add casts to bf16
                kvt16 = small.tile([DE, DE], bf16)
                if ngrp == 4:
                    s01 = small.tile([DE, DE], f32)
                    nc.vector.tensor_add(s01, kvt_all[:, 0, :], kvt_all[:, 1, :])
                    s23 = small.tile([DE, DE], f32)
                    nc.vector.tensor_add(s23, kvt_all[:, 2, :], kvt_all[:, 3, :])
                    nc.vector.tensor_add(kvt16, s01, s23)
                elif ngrp == 2:
                    nc.vector.tensor_add(kvt16, kvt_all[:, 0, :], kvt_all[:, 1, :])
                else:
                    nc.vector.tensor_copy(kvt16, kvt_all[:, 0, :])
                mm_ps = mmpsp.tile([DE, DIN], f32)
                nc.tensor.matmul(mm_ps, lhsT=kvt16, rhs=sb_wot)
                nc.vector.tensor_copy(mmat[:DE, :], mm_ps)

            # ---- Phase C: out = sigmoid(e1 @ M + bO) ----
            with (
                tc.tile_pool(name="ops", bufs=2, space="PSUM") as opsp,
                tc.tile_pool(name="osb", bufs=2) as osbp,
            ):
                for j in range(n_chunks):
                    osb = osbp.tile([128, 4, DIN], f32)
                    for t in range(4):
                        tt = j * 4 + t
                        o_ps = opsp.tile([128, DIN], f32)
                        nc.tensor.matmul(
                            o_ps,
                            lhsT=e1t[: DE + 1, tt * 128 : (tt + 1) * 128],
                            rhs=mmat[: DE + 1, :],
                        )
                        nc.scalar.activation(
                            osb[:, t, :], o_ps, mybir.ActivationFunctionType.Sigmoid
                        )
                    nc.sync.dma_start(
                        out=out.ap()[j * 512 : (j + 1) * 512, :].rearrange(
                            "(t p) o -> p t o", p=128
                        ),
                        in_=osb,
                    )
    nc.compile()
    return nc


def make_in_maps(x, W1, b1, W2, b2, W3, b3, WO, bO, rows=ROWS, n_cores=N_CORES):
    x = np.asarray(x, dtype=np.float32)
    total = x.shape[0] * x.shape[1]
    xt_full = np.ascontiguousarray(x.reshape(total, DIN).T)  # [512, total]
    shared = {
        "w1t": np.ascontiguousarray(np.asarray(W1, np.float32).T).astype(BF16),
        "w23t": np.ascontiguousarray(
            np.concatenate(
                [np.asarray(W2, np.float32).T, np.asarray(W3, np.float32).T], axis=1
            )
        ).astype(BF16),
        "wot": np.ascontiguousarray(np.asarray(WO, np.float32).T).astype(BF16),
        "b1": np.ascontiguousarray(np.asarray(b1, np.float32).reshape(DE, 1)),
        "b23": np.ascontiguousarray(
            np.concatenate([np.asarray(b2, np.float32), np.asarray(b3, np.float32)])
        ).reshape(1, 2 * DE),
        "bo": np.asarray(bO, np.float32).reshape(1, DIN).astype(BF16),
    }
    in_maps = []
    for c in range(n_cores):
        m = dict(shared)
        m["xt"] = np.ascontiguousarray(xt_full[:, c * rows : (c + 1) * rows])
        in_maps.append(m)
    return in_maps


def kernel(x, W1, b1, W2, b2, W3, b3, WO, bO):
    global LAST_RESULT
    if "nc" not in _NC_CACHE:
        _NC_CACHE["nc"] = build_nc()
    nc = _NC_CACHE["nc"]
    in_maps = make_in_maps(x, W1, b1, W2, b2, W3, b3, WO, bO)
    res = run_bass_kernel_spmd(
        nc,
        in_maps,
        core_ids=list(range(N_CORES)),
        trace=TRACE,
        **TRACE_KWARGS,
    )
    LAST_RESULT = res
    full = np.concatenate(
        [res.results[c]["out"] for c in range(N_CORES)], axis=0
    )  # [16384, 512] f32
    return full.reshape(BATCH, SEQ, DIN)


# revision 6
# speedup vs baseline: 1.1261x; 1.1261x over previous
# Trainium2 Bass kernel for nn_Attention3 (unnormalized linear attention).
#
# Math: e_i = x @ W_i.T + b_i (i=1,2,3);  out = sigmoid((e1 @ e2.T @ e3) @ WO.T + bO)
# Since there is no softmax, (e1 @ e2.T) @ e3 == e1 @ (e2.T @ e3) where
# KV = e2.T @ e3 is only [64, 64].  The kernel is therefore memory-bound:
# read x once, write out once.
#
# Sharding: the flattened [B*S, 512] = [16384, 512] rows are split into 8
# contiguous chunks of 2048 rows (cores 0-3 <- batch 0, cores 4-7 <- batch 1).
# Each core computes its partial KV^T = e3_c.T @ e2_c over its rows, the four
# cores of a batch AllGather+sum their partials, then each core finishes
# out = sigmoid(e1 @ (KV @ WO.T) + bO) for its rows.
#
# Precision/layout: x arrives host-transposed ([512, rows] f32) so the
# contraction dim sits on SBUF partitions, and is DMA-cast to float32r
# (tf32-grade, full PE rate at free-dim >= 256).  Projections are computed
# transposed ([64|128, 512] out tiles, N=512) in f32r; e2|e3 is bias-added and
# PE-transposed back to natural layout in full f32 for the KV^T accumulation.
# The final e1 @ (KV @ WO.T) matmul runs in f32r at N=512.

import numpy as np

import concourse.mybir as mybir
import concourse.tile as tile
from concourse import bacc
from concourse.bass_utils import run_bass_kernel_spmd
from concourse.masks import make_identity

BATCH = 2
SEQ = 8192
DIN = 512
DE = 64
N_CORES = 8
ROWS = (BATCH * SEQ) // N_CORES  # 2048 rows per core

TRACE = False
TRACE_KWARGS = {}
LAST_RESULT = None

_NC_CACHE = {}


def build_nc(rows=ROWS, n_cores=N_CORES):
    f32 = mybir.dt.float32
    f32r = mybir.dt.float32r

    half = n_cores // 2
    groups = [list(range(half)), list(range(half, n_cores))]
    ngrp = half

    assert rows % 512 == 0
    n_chunks = rows // 512

    nc = bacc.Bacc(None, target_bir_lowering=False, debug=False, num_devices=n_cores)

    xt = nc.dram_tensor("xt", [DIN, rows], f32, kind="ExternalInput")
    w1t = nc.dram_tensor("w1t", [DIN, DE], f32, kind="ExternalInput")
    w23t = nc.dram_tensor("w23t", [DIN, 2 * DE], f32, kind="ExternalInput")
    wot = nc.dram_tensor("wot", [DE, DIN], f32, kind="ExternalInput")
    b1 = nc.dram_tensor("b1", [DE, 1], f32, kind="ExternalInput")
    b23 = nc.dram_tensor("b23", [2 * DE, 1], f32, kind="ExternalInput")
    bo = nc.dram_tensor("bo", [1, DIN], f32, kind="ExternalInput")
    out = nc.dram_tensor("out", [rows, DIN], f32, kind="ExternalOutput")

    xt_t = xt.ap().rearrange("(kt p) s -> p kt s", p=128)  # [128, 4, rows]

    with tile.TileContext(nc) as tc:
        with (
            tc.tile_pool(name="consts", bufs=1) as consts,
            tc.tile_pool(name="persist", bufs=1) as persist,
            tc.tile_pool(name="kvps", bufs=1, space="PSUM") as kvps,
            tc.tile_pool(name="dram", bufs=1, space="DRAM") as dram,
        ):
            # weights, DMA-cast f32 -> f32r (SWDGE)
            sb_w1t = consts.tile([128, 4, DE], f32r)
            nc.gpsimd.dma_start(out=sb_w1t, in_=w1t.ap().rearrange("(kt p) d -> p kt d", p=128))
            sb_w23t = consts.tile([128, 4, 2 * DE], f32r)
            nc.gpsimd.dma_start(out=sb_w23t, in_=w23t.ap().rearrange("(kt p) d -> p kt d", p=128))
            sb_wot = consts.tile([DE, DIN], f32r)
            nc.gpsimd.dma_start(out=sb_wot, in_=wot.ap())
            sb_b1 = consts.tile([DE, 1], f32)
            nc.sync.dma_start(out=sb_b1, in_=b1.ap())
            sb_b23 = consts.tile([2 * DE, 1], f32)
            nc.sync.dma_start(out=sb_b23, in_=b23.ap())
            identity = consts.tile([128, 128], f32)
            make_identity(nc, identity[:, :])

            # e1^T for all local rows, with a row of ones at partition DE so the
            # final matmul folds in the output bias (lhsT K = DE+1).
            e1t = persist.tile([128, rows], f32r)
            ones_row = consts.tile([1, rows], f32)
            nc.vector.memset(ones_row, 1.0)
            nc.vector.tensor_copy(e1t[DE : DE + 1, :], ones_row)
            # M = KV @ WO.T in rows 0..63, bO in row DE.
            mmat = persist.tile([128, DIN], f32r)
            nc.gpsimd.dma_start(out=mmat[DE : DE + 1, :], in_=bo.ap())

            kvt_ps = kvps.tile([DE, DE], f32)  # accumulates e3^T @ e2 over all tiles

            # ---- Phase A: load x^T (cast to f32r), project, partial KV^T ----
            with (
                tc.tile_pool(name="xr", bufs=2) as xrp,
                tc.tile_pool(name="e23tps", bufs=2, space="PSUM") as e23tpsp,
                tc.tile_pool(name="e23tsb", bufs=2) as e23tsbp,
                tc.tile_pool(name="trps", bufs=2, space="PSUM") as trpsp,
                tc.tile_pool(name="e23n", bufs=3) as e23np,
                tc.tile_pool(name="e1ps", bufs=2, space="PSUM") as e1psp,
            ):
                for j in range(n_chunks):
                    xr = xrp.tile([128, 4, 512], f32r)
                    nc.gpsimd.dma_start(out=xr, in_=xt_t[:, :, j * 512 : (j + 1) * 512])

                    # e23T = [W2;W3] @ x^T  -> [128, 512] (d on partitions)
                    e23t_ps = e23tpsp.tile([128, 512], f32)
                    for kt in range(4):
                        nc.tensor.matmul(
                            e23t_ps,
                            lhsT=sb_w23t[:, kt, :],
                            rhs=xr[:, kt, :],
                            start=(kt == 0),
                            stop=(kt == 3),
                        )
                    e23t_sb = e23tsbp.tile([128, 512], f32)
                    nc.scalar.activation(
                        e23t_sb,
                        e23t_ps,
                        mybir.ActivationFunctionType.Identity,
                        bias=sb_b23,
                        scale=1.0,
                    )

                    # e1T = W1 @ x^T -> [64, 512], + b1, kept f32r for phase C
                    e1_ps = e1psp.tile([DE, 512], f32)
                    for kt in range(4):
                        nc.tensor.matmul(
                            e1_ps,
                            lhsT=sb_w1t[:, kt, :],
                            rhs=xr[:, kt, :],
                            start=(kt == 0),
                            stop=(kt == 3),
                        )
                    nc.vector.tensor_scalar_add(
                        e1t[:DE, j * 512 : (j + 1) * 512], e1_ps, sb_b1
                    )

                    # transpose e23T back to natural layout (full f32) and
                    # accumulate KV^T = e3^T @ e2
                    for t in range(4):
                        tt = j * 4 + t
                        tr_ps = trpsp.tile([128, 128], f32)
                        nc.tensor.transpose(
                            tr_ps, e23t_sb[:, t * 128 : (t + 1) * 128], identity[:, :]
                        )
                        e23n = e23np.tile([128, 128], f32)
                        nc.vector.tensor_copy(e23n, tr_ps)
                        nc.tensor.matmul(
                            kvt_ps,
                            lhsT=e23n[:, DE:],
                            rhs=e23n[:, :DE],
                            start=(tt == 0),
                            stop=(tt == 4 * n_chunks - 1),
                        )

            # ---- Collective: AllGather partial KV^T within the batch group ----
            with (
                tc.tile_pool(name="small", bufs=1) as small,
                tc.tile_pool(name="mmps", bufs=1, space="PSUM") as mmpsp,
            ):
                kvt_sb = small.tile([DE, DE], f32)
                nc.vector.tensor_copy(kvt_sb, kvt_ps)
                cc_in = dram.tile([DE, DE], f32)
                cc_out = dram.tile([ngrp, DE, DE], f32)
                nc.gpsimd.dma_start(out=cc_in[:, :], in_=kvt_sb)
                nc.gpsimd.collective_compute(
                    "AllGather",
                    mybir.AluOpType.bypass,
                    replica_groups=groups,
                    ins=[cc_in[:, :]],
                    outs=[cc_out[:, :, :]],
                )
                kvt_all = small.tile([DE, ngrp, DE], f32)
                nc.sync.dma_start(
                    out=kvt_all, in_=cc_out[:, :, :].rearrange("r p d -> p r d")
                )
                # tree-sum the ngrp partials; final add casts to f32r
                kvt_r = small.tile([DE, DE], f32r)
                if ngrp == 4:
                    s01 = small.tile([DE, DE], f32)
                    nc.vector.tensor_add(s01, kvt_all[:, 0, :], kvt_all[:, 1, :])
                    s23 = small.tile([DE, DE], f32)
                    nc.vector.tensor_add(s23, kvt_all[:, 2, :], kvt_all[:, 3, :])
                    nc.vector.tensor_add(kvt_r, s01, s23)
                elif ngrp == 2:
                    nc.vector.tensor_add(kvt_r, kvt_all[:, 0, :], kvt_all[:, 1, :])
                else:
                    nc.vector.tensor_copy(kvt_r, kvt_all[:, 0, :])
                mm_ps = mmpsp.tile([DE, DIN], f32)
                nc.tensor.matmul(mm_ps, lhsT=kvt_r, rhs=sb_wot)
                nc.vector.tensor_copy(mmat[:DE, :], mm_ps)

            # ---- Phase C: out = sigmoid(e1 @ M + bO) ----
            with (
                tc.tile_pool(name="ops", bufs=2, space="PSUM") as opsp,
                tc.tile_pool(name="osb", bufs=2) as osbp,
            ):
                for j in range(n_chunks):
                    osb = osbp.tile([128, 4, DIN], f32)
                    for t in range(4):
                        tt = j * 4 + t
                        o_ps = opsp.tile([128, DIN], f32)
                        nc.tensor.matmul(
                            o_ps,
                            lhsT=e1t[: DE + 1, tt * 128 : (tt + 1) * 128],
                            rhs=mmat[: DE + 1, :],
                        )
                        nc.scalar.activation(
                            osb[:, t, :], o_ps, mybir.ActivationFunctionType.Sigmoid
                        )
                    nc.sync.dma_start(
                        out=out.ap()[j * 512 : (j + 1) * 512, :].rearrange(
                            "(t p) o -> p t o", p=128
                        ),
                        in_=osb,
                    )
    nc.compile()
    return nc


def make_in_maps(x, W1, b1, W2, b2, W3, b3, WO, bO, rows=ROWS, n_cores=N_CORES):
    x = np.asarray(x, dtype=np.float32)
    total = x.shape[0] * x.shape[1]
    xt_full = np.ascontiguousarray(x.reshape(total, DIN).T)  # [512, total]
    shared = {
        "w1t": np.ascontiguousarray(np.asarray(W1, np.float32).T),
        "w23t": np.ascontiguousarray(
            np.concatenate(
                [np.asarray(W2, np.float32).T, np.asarray(W3, np.float32).T], axis=1
            )
        ),
        "wot": np.ascontiguousarray(np.asarray(WO, np.float32).T),
        "b1": np.ascontiguousarray(np.asarray(b1, np.float32).reshape(DE, 1)),
        "b23": np.ascontiguousarray(
            np.concatenate([np.asarray(b2, np.float32), np.asarray(b3, np.float32)])
        ).reshape(2 * DE, 1),
        "bo": np.ascontiguousarray(np.asarray(bO, np.float32).reshape(1, DIN)),
    }
    in_maps = []
    for c in range(n_cores):
        m = dict(shared)
        m["xt"] = np.ascontiguousarray(xt_full[:, c * rows : (c + 1) * rows])
        in_maps.append(m)
    return in_maps


def kernel(x, W1, b1, W2, b2, W3, b3, WO, bO):
    global LAST_RESULT
    if "nc" not in _NC_CACHE:
        _NC_CACHE["nc"] = build_nc()
    nc = _NC_CACHE["nc"]
    in_maps = make_in_maps(x, W1, b1, W2, b2, W3, b3, WO, bO)
    res = run_bass_kernel_spmd(
        nc,
        in_maps,
        core_ids=list(range(N_CORES)),
        trace=TRACE,
        **TRACE_KWARGS,
    )
    LAST_RESULT = res
    full = np.concatenate(
        [res.results[c]["out"] for c in range(N_CORES)], axis=0
    )  # [16384, 512] f32
    return full.reshape(BATCH, SEQ, DIN)
